# revision 8
# baseline (speedup 1.0000x reference)
"""Trainium2 Bass kernel for nn_AttentionBlock (sparse_attention).

Reference computation per batch b (channels-first x[b]: [C=512, T=4096]):
    xt = x[b].T                                  # [T, C]
    q = xt @ Wq.T + bq ; k = xt @ Wk.T + bk      # [T, 512]
    v = xt @ Wv.T + bv                           # [T, 512]
    S = q @ k.T / sqrt(512), causal (j <= i)     # [T, T]
    P = softmax(S, axis=QUERY i)  (per-column normalization)
    act = P @ v                                  # [T, 512]
    out[b] = concat(x[b], act.T, axis=0)         # [1024, T]

Sharding: pure data-parallel over batch B=8 across the 8 NeuronCores
(one batch per core, no collectives).

Per-core algorithm (all matmuls fp16 with f32 PSUM accumulation):
  1. QKV projections from x (host-cast to fp16), producing
     Q^T,K^T: [512, T] (head-dim on partitions) and V: [T, 512]
     (time on partitions).  1/sqrt(512) score scale folded into
     Wq,bq,Wk,bk on host (split as 512**-0.25 on each side).
  2. ST[j,i] = K^T.T @ Q^T strips (j-chunk of 128 rows at a time,
     i from the diagonal to T).  Column-softmax over i = free-axis
     ops: additive causal mask on the diagonal 128x128, then
     exp(s - 4) on ScalarE with accum_out giving the row sums Z_j.
     P~ = exp(ST - 4) stored to DRAM scratch as fp16.
  3. V rows scaled by 1/Z_j (folds softmax denominator into V).
  4. act^T[v,i] = sum_j V'[j,v] * P~[j,i]: PSUM-accumulated matmuls
     over j-chunks, streaming P~ tiles back from DRAM.
  5. out rows 0..511 are a DRAM->DRAM copy of x[b]; rows 512..1023
     get act^T.
"""

import math

import numpy as np

import concourse.bass as bass
import concourse.mybir as mybir
from concourse import bacc, tile
from concourse.bass_utils import run_bass_kernel_spmd

P = 128
C = 512
T = 4096
KDIM = 512
VDIM = 512
NCC = C // P      # 4 contraction chunks over channels
NKK = KDIM // P   # 4 chunks of head dim
NTC = T // P      # 32 time chunks of 128
NIB = T // 512    # 8 i-blocks of 512
F16 = mybir.dt.float16
F32 = mybir.dt.float32
EXP_SHIFT = -4.0  # constant logit shift: softmax-invariant, keeps exp in fp16 range
MASK_NEG = -10000.0

_CACHE = {}


def _ts(i, size):
    return slice(i * size, (i + 1) * size)


def build_nc():
    nc = bacc.Bacc(
        "TRN2",
        target_bir_lowering=False,
        debug=False,
        num_devices=8,
    )

    x16_d = nc.declare_dram_parameter("x16", [C, T], F16, isOutput=False)
    x32_d = nc.declare_dram_parameter("x32", [C, T], F32, isOutput=False)
    wqt_d = nc.declare_dram_parameter("wqt", [C, KDIM], F16, isOutput=False)
    wkt_d = nc.declare_dram_parameter("wkt", [C, KDIM], F16, isOutput=False)
    wvt_d = nc.declare_dram_parameter("wvt", [C, VDIM], F16, isOutput=False)
    bq_d = nc.declare_dram_parameter("bq", [P, NKK], F32, isOutput=False)
    bk_d = nc.declare_dram_parameter("bk", [P, NKK], F32, isOutput=False)
    bv_d = nc.declare_dram_parameter("bv", [P, VDIM], F32, isOutput=False)
    mask_d = nc.declare_dram_parameter("mask", [P, P], F32, isOutput=False)
    out_d = nc.declare_dram_parameter("out", [C + VDIM, T], F32, isOutput=True)

    with tile.TileContext(nc) as tc:
        from contextlib import ExitStack

        with ExitStack() as ctx:
            singles = ctx.enter_context(tc.tile_pool(name="singles", bufs=1))

            def single(shape, dtype, tag):
                return singles.tile(shape, dtype, name=tag, tag=tag)

            x16_s = [single([P, T], F16, f"x16s{c}") for c in range(NCC)]
            wq_s = [single([P, KDIM], F16, f"wqs{c}") for c in range(NCC)]
            wk_s = [single([P, KDIM], F16, f"wks{c}") for c in range(NCC)]
            wv_s = [single([P, VDIM], F16, f"wvs{c}") for c in range(NCC)]
            bq_s = single([P, NKK], F32, "bqs")
            bk_s = single([P, NKK], F32, "bks")
            bv_s = single([P, VDIM], F32, "bvs")
            mask_s = single([P, P], F32, "masks")
            qt_s = [single([P, T], F16, f"qts{k}") for k in range(NKK)]
            kt_s = [single([P, T], F16, f"kts{k}") for k in range(NKK)]
            v_s = [single([P, VDIM], F16, f"vs{t}") for t in range(NTC)]
            zr_s = single([P, NTC], F32, "zrs")
            zero_s = single([P, 384], F16, "zeros")
            expshift_s = single([P, 1], F32, "expshift")
            nc.vector.memset(expshift_s, EXP_SHIFT)

            # ---- input DMAs + x copy-through (DRAM->DRAM) ----
            for c in range(NCC):
                nc.sync.dma_start(out=x16_s[c], in_=x16_d[_ts(c, P), :])
                nc.sync.dma_start(out=wq_s[c], in_=wqt_d[_ts(c, P), :])
                nc.sync.dma_start(out=wk_s[c], in_=wkt_d[_ts(c, P), :])
                nc.sync.dma_start(out=wv_s[c], in_=wvt_d[_ts(c, P), :])
                nc.sync.dma_start(out=out_d[_ts(c, P), :], in_=x32_d[_ts(c, P), :])
            nc.sync.dma_start(out=bq_s, in_=bq_d[:, :])
            nc.sync.dma_start(out=bk_s, in_=bk_d[:, :])
            nc.sync.dma_start(out=bv_s, in_=bv_d[:, :])
            nc.sync.dma_start(out=mask_s, in_=mask_d[:, :])
            nc.vector.memset(zero_s, 0.0)

            # P~ strips in DRAM scratch, one tile per j-chunk so Tile's
            # dependency tracking stays per-strip.
            ptdram = ctx.enter_context(
                tc.tile_pool(name="ptdram", bufs=1, space="DRAM")
            )
            pt_d = [
                ptdram.tile([P, T], F16, name=f"pt{j}", tag=f"pt{j}")
                for j in range(NTC)
            ]

            # ---- Phase QKV ----
            qkv_ps_cm = tc.tile_pool(name="qkv_ps", bufs=4, space="PSUM")
            qkv_ps = qkv_ps_cm.__enter__()
            for dst, w_s, b_s in ((qt_s, wq_s, bq_s), (kt_s, wk_s, bk_s)):
                for kk in range(NKK):
                    for ib in range(NIB):
                        ps = qkv_ps.tile([P, 512], F32, tag="qkvps", name="ps_qk")
                        for c in range(NCC):
                            nc.tensor.matmul(
                                ps,
                                lhsT=w_s[c][:, _ts(kk, P)],
                                rhs=x16_s[c][:, _ts(ib, 512)],
                                start=(c == 0),
                                stop=(c == NCC - 1),
                            )
                        nc.scalar.activation(
                            dst[kk][:, _ts(ib, 512)],
                            ps,
                            mybir.ActivationFunctionType.Identity,
                            bias=b_s[:, kk : kk + 1],
                            scale=1.0,
                        )
            for t in range(NTC):
                ps = qkv_ps.tile([P, 512], F32, tag="qkvps", name="ps_v")
                for c in range(NCC):
                    nc.tensor.matmul(
                        ps,
                        lhsT=x16_s[c][:, _ts(t, P)],
                        rhs=wv_s[c],
                        start=(c == 0),
                        stop=(c == NCC - 1),
                    )
                nc.vector.tensor_add(v_s[t], ps, bv_s)
            qkv_ps_cm.__exit__(None, None, None)

            # ---- Phase 1: score strips + column softmax stats ----
            s_ps_cm = tc.tile_pool(name="s_ps", bufs=4, space="PSUM")
            s_ps = s_ps_cm.__enter__()
            ptb_pool = ctx.enter_context(tc.tile_pool(name="ptb", bufs=4))
            zp_pool = ctx.enter_context(tc.tile_pool(name="zp", bufs=4))

            for jc in range(NTC):
                i0 = P * jc
                a0 = 512 * (jc // 4)
                r = jc % 4
                if r > 0:
                    # zero the never-written corner so phase 2 reads are clean
                    nc.sync.dma_start(
                        out=pt_d[jc][:, a0:i0], in_=zero_s[:, 0 : P * r]
                    )
                starts = [i0] + list(range(a0 + 512, T, 512))
                nch = len(starts)
                zp = zp_pool.tile([P, NIB], F32, tag="zp", name="zp")
                for ci, a in enumerate(starts):
                    b = a0 + 512 * (ci + 1)
                    w = b - a
                    ps = s_ps.tile([P, 512], F32, tag="sps", name="ps_s")
                    for kk in range(NKK):
                        nc.tensor.matmul(
                            ps[:, 0:w],
                            lhsT=kt_s[kk][:, _ts(jc, P)],
                            rhs=qt_s[kk][:, a:b],
                            start=(kk == 0),
                            stop=(kk == NKK - 1),
                        )
                    if ci == 0:
                        nc.vector.tensor_add(ps[:, 0:P], ps[:, 0:P], mask_s)
                    ptb = ptb_pool.tile([P, 512], F16, tag="ptb", name="ptb")
                    nc.scalar.activation(
                        ptb[:, 0:w],
                        ps[:, 0:w],
                        mybir.ActivationFunctionType.Exp,
                        bias=expshift_s[:, 0:1],
                        scale=1.0,
                        accum_out=zp[:, ci : ci + 1],
                    )
                    nc.sync.dma_start(out=pt_d[jc][:, a:b], in_=ptb[:, 0:w])
                z = zp_pool.tile([P, 1], F32, tag="zf", name="z")
                nc.vector.reduce_sum(z, zp[:, 0:nch], axis=mybir.AxisListType.X)
                nc.vector.reciprocal(zr_s[:, jc : jc + 1], z)
                # fold 1/Z_j into V rows (partition j)
                nc.vector.tensor_scalar_mul(v_s[jc], v_s[jc], zr_s[:, jc : jc + 1])

            s_ps_cm.__exit__(None, None, None)

            # ---- Phase 2: act^T = V'^T @ P~ ----
            act_ps = ctx.enter_context(
                tc.tile_pool(name="act_ps", bufs=1, space="PSUM")
            )
            pti_pool = ctx.enter_context(tc.tile_pool(name="pti", bufs=6))
            ob_pool = ctx.enter_context(tc.tile_pool(name="ob", bufs=4))
            for ib in range(NIB):
                njc = 4 * (ib + 1)
                pss = [
                    act_ps.tile([P, 512], F32, tag=f"aps{v}", name=f"aps{v}")
                    for v in range(4)
                ]
                for jc in range(njc):
                    pti = pti_pool.tile([P, 512], F16, tag="pti", name="pti")
                    nc.sync.dma_start(out=pti, in_=pt_d[jc][:, _ts(ib, 512)])
                    for vc in range(4):
                        nc.tensor.matmul(
                            pss[vc],
                            lhsT=v_s[jc][:, _ts(vc, P)],
                            rhs=pti,
                            start=(jc == 0),
                            stop=(jc == njc - 1),
                        )
                for vc in range(4):
                    ob = ob_pool.tile([P, 512], F32, tag="ob", name="ob")
                    if vc % 2 == 0:
                        nc.scalar.copy(ob, pss[vc])
                    else:
                        nc.vector.tensor_copy(ob, pss[vc])
                    nc.sync.dma_start(
                        out=out_d[C + vc * P : C + (vc + 1) * P, _ts(ib, 512)],
                        in_=ob,
                    )

    nc.compile()
    return nc


def _host_inputs(x, Wq, bq, Wk, bk, Wv, bv):
    c4 = float(C) ** 0.25
    wqt = np.ascontiguousarray(Wq.T / c4).astype(np.float16)
    wkt = np.ascontiguousarray(Wk.T / c4).astype(np.float16)
    wvt = np.ascontiguousarray(Wv.T).astype(np.float16)
    bq_h = np.ascontiguousarray((bq / c4).reshape(NKK, P).T).astype(np.float32)
    bk_h = np.ascontiguousarray((bk / c4).reshape(NKK, P).T).astype(np.float32)
    bv_h = np.ascontiguousarray(np.tile(bv.astype(np.float32), (P, 1)))
    r = np.arange(P)
    mask = np.where(r[None, :] >= r[:, None], 0.0, MASK_NEG).astype(np.float32)
    in_maps = []
    for b in range(x.shape[0]):
        xb = np.ascontiguousarray(x[b]).astype(np.float32)
        in_maps.append(
            {
                "x16": xb.astype(np.float16),
                "x32": xb,
                "wqt": wqt,
                "wkt": wkt,
                "wvt": wvt,
                "bq": bq_h,
                "bk": bk_h,
                "bv": bv_h,
                "mask": mask,
            }
        )
    return in_maps


def kernel(x, Wq, bq, Wk, bk, Wv, bv, _trace=False):
    import time as _time

    x = np.asarray(x, dtype=np.float32)
    if "nc" not in _CACHE:
        t0 = _time.time()
        _CACHE["nc"] = build_nc()
        print(f"[kernel] build_nc done in {_time.time() - t0:.1f}s", flush=True)
    nc = _CACHE["nc"]
    in_maps = _host_inputs(
        x,
        np.asarray(Wq, np.float32),
        np.asarray(bq, np.float32),
        np.asarray(Wk, np.float32),
        np.asarray(bk, np.float32),
        np.asarray(Wv, np.float32),
        np.asarray(bv, np.float32),
    )
    t0 = _time.time()
    res = run_bass_kernel_spmd(
        nc, in_maps, core_ids=list(range(8)), trace=_trace
    )
    print(f"[kernel] run done in {_time.time() - t0:.1f}s", flush=True)
    _CACHE["last_result"] = res
    out = np.stack([r["out"] for r in res.results]).astype(np.float32)
    return out


# revision 15
# speedup vs baseline: 1.1785x; 1.1785x over previous
"""Trainium2 Bass kernel for nn_AttentionBlock (sparse_attention).

Reference computation per batch b (channels-first x[b]: [C=512, T=4096]):
    xt = x[b].T                                  # [T, C]
    q = xt @ Wq.T + bq ; k = xt @ Wk.T + bk      # [T, 512]
    v = xt @ Wv.T + bv                           # [T, 512]
    S = q @ k.T / sqrt(512), causal (j <= i)     # [T, T]
    P = softmax(S, axis=QUERY i)  (per-column normalization)
    act = P @ v                                  # [T, 512]
    out[b] = concat(x[b], act.T, axis=0)         # [1024, T]

Sharding: pure data-parallel over batch B=8 across the 8 NeuronCores
(one batch per core, no collectives).

Per-core algorithm (all matmuls fp16 with f32 PSUM accumulation):
  1. QKV projections from x (host-cast to fp16), producing
     Q^T,K^T: [512, T] (head-dim on partitions) and V: [T, 512]
     (time on partitions).  1/sqrt(512) score scale folded into
     Wq,bq,Wk,bk on host (split as 512**-0.25 on each side).
  2. ST[j,i] = K^T.T @ Q^T strips (j-chunk of 128 rows at a time,
     i from the diagonal to T).  Column-softmax over i = free-axis
     ops: additive causal mask on the diagonal 128x128, then
     exp(s - 4) on ScalarE with accum_out giving the row sums Z_j.
     P~ = exp(ST - 4) stored to DRAM scratch as fp16.
  3. V rows scaled by 1/Z_j (folds softmax denominator into V).
  4. act^T[v,i] = sum_j V'[j,v] * P~[j,i]: PSUM-accumulated matmuls
     over j-chunks, streaming P~ tiles back from DRAM.
  5. out rows 0..511 are a DRAM->DRAM copy of x[b]; rows 512..1023
     get act^T.
"""

import math

import numpy as np

import concourse.bass as bass
import concourse.mybir as mybir
from concourse import bacc, tile
from concourse.bass_utils import run_bass_kernel_spmd

P = 128
C = 512
T = 4096
KDIM = 512
VDIM = 512
NCC = C // P      # 4 contraction chunks over channels
NKK = KDIM // P   # 4 chunks of head dim
NTC = T // P      # 32 time chunks of 128
NIB = T // 512    # 8 i-blocks of 512
F16 = mybir.dt.float16
F32 = mybir.dt.float32
EXP_SHIFT = -4.0  # constant logit shift: softmax-invariant, keeps exp in fp16 range
MASK_NEG = -10000.0

_CACHE = {}


def _ts(i, size):
    return slice(i * size, (i + 1) * size)


def build_nc():
    nc = bacc.Bacc(
        "TRN2",
        target_bir_lowering=False,
        debug=False,
        num_devices=8,
    )

    x16_d = nc.declare_dram_parameter("x16", [C, T], F16, isOutput=False)
    x32_d = nc.declare_dram_parameter("x32", [C, T], F32, isOutput=False)
    wqt_d = nc.declare_dram_parameter("wqt", [C, KDIM], F16, isOutput=False)
    wkt_d = nc.declare_dram_parameter("wkt", [C, KDIM], F16, isOutput=False)
    wvt_d = nc.declare_dram_parameter("wvt", [C, VDIM], F16, isOutput=False)
    bq_d = nc.declare_dram_parameter("bq", [P, NKK], F32, isOutput=False)
    bk_d = nc.declare_dram_parameter("bk", [P, NKK], F32, isOutput=False)
    bv_d = nc.declare_dram_parameter("bv", [P, VDIM], F32, isOutput=False)
    mask_d = nc.declare_dram_parameter("mask", [P, P], F32, isOutput=False)
    out_d = nc.declare_dram_parameter("out", [C + VDIM, T], F32, isOutput=True)

    with tile.TileContext(nc) as tc:
        from contextlib import ExitStack

        with ExitStack() as ctx:
            singles = ctx.enter_context(tc.tile_pool(name="singles", bufs=1))

            def single(shape, dtype, tag):
                return singles.tile(shape, dtype, name=tag, tag=tag)

            x16_s = [single([P, T], F16, f"x16s{c}") for c in range(NCC)]
            wq_s = [single([P, KDIM], F16, f"wqs{c}") for c in range(NCC)]
            wk_s = [single([P, KDIM], F16, f"wks{c}") for c in range(NCC)]
            wv_s = [single([P, VDIM], F16, f"wvs{c}") for c in range(NCC)]
            bq_s = single([P, NKK], F32, "bqs")
            bk_s = single([P, NKK], F32, "bks")
            bv_s = single([P, VDIM], F32, "bvs")
            mask_s = single([P, P], F32, "masks")
            qt_s = [single([P, T], F16, f"qts{k}") for k in range(NKK)]
            kt_s = [single([P, T], F16, f"kts{k}") for k in range(NKK)]
            v_s = [single([P, VDIM], F16, f"vs{t}") for t in range(NTC)]
            zr_s = single([P, NTC], F32, "zrs")
            zero_s = single([P, 384], F16, "zeros")
            expshift_s = single([P, 1], F32, "expshift")
            nc.vector.memset(expshift_s, EXP_SHIFT)

            # ---- input DMAs + x copy-through (DRAM->DRAM) ----
            for c in range(NCC):
                nc.sync.dma_start(out=x16_s[c], in_=x16_d[_ts(c, P), :])
                nc.sync.dma_start(out=wq_s[c], in_=wqt_d[_ts(c, P), :])
                nc.sync.dma_start(out=wk_s[c], in_=wkt_d[_ts(c, P), :])
                nc.sync.dma_start(out=wv_s[c], in_=wvt_d[_ts(c, P), :])
                nc.sync.dma_start(out=out_d[_ts(c, P), :], in_=x32_d[_ts(c, P), :])
            nc.sync.dma_start(out=bq_s, in_=bq_d[:, :])
            nc.sync.dma_start(out=bk_s, in_=bk_d[:, :])
            nc.sync.dma_start(out=bv_s, in_=bv_d[:, :])
            nc.sync.dma_start(out=mask_s, in_=mask_d[:, :])
            nc.vector.memset(zero_s, 0.0)

            # P~ strips in DRAM scratch, one tile per j-chunk so Tile's
            # dependency tracking stays per-strip.
            ptdram = ctx.enter_context(
                tc.tile_pool(name="ptdram", bufs=1, space="DRAM")
            )
            pt_d = [
                ptdram.tile([P, T], F16, name=f"pt{j}", tag=f"pt{j}")
                for j in range(NTC)
            ]

            # ---- Phase QKV ----
            qkv_ps_cm = tc.tile_pool(name="qkv_ps", bufs=4, space="PSUM")
            qkv_ps = qkv_ps_cm.__enter__()
            for dst, w_s, b_s in ((qt_s, wq_s, bq_s), (kt_s, wk_s, bk_s)):
                for kk in range(NKK):
                    for ib in range(NIB):
                        ps = qkv_ps.tile([P, 512], F32, tag="qkvps", name="ps_qk")
                        for c in range(NCC):
                            nc.tensor.matmul(
                                ps,
                                lhsT=w_s[c][:, _ts(kk, P)],
                                rhs=x16_s[c][:, _ts(ib, 512)],
                                start=(c == 0),
                                stop=(c == NCC - 1),
                            )
                        nc.scalar.activation(
                            dst[kk][:, _ts(ib, 512)],
                            ps,
                            mybir.ActivationFunctionType.Identity,
                            bias=b_s[:, kk : kk + 1],
                            scale=1.0,
                        )
            for t in range(NTC):
                ps = qkv_ps.tile([P, 512], F32, tag="qkvps", name="ps_v")
                for c in range(NCC):
                    nc.tensor.matmul(
                        ps,
                        lhsT=x16_s[c][:, _ts(t, P)],
                        rhs=wv_s[c],
                        start=(c == 0),
                        stop=(c == NCC - 1),
                    )
                nc.vector.tensor_add(v_s[t], ps, bv_s)
            qkv_ps_cm.__exit__(None, None, None)

            # ---- Phase 1: score strips + column softmax stats ----
            s_ps_cm = tc.tile_pool(name="s_ps", bufs=4, space="PSUM")
            s_ps = s_ps_cm.__enter__()
            ptb_pool = ctx.enter_context(tc.tile_pool(name="ptb", bufs=4))
            zp_pool = ctx.enter_context(tc.tile_pool(name="zp", bufs=4))

            for jc in range(NTC):
                i0 = P * jc
                a0 = 512 * (jc // 4)
                r = jc % 4
                if r > 0:
                    # zero the never-written corner so phase 2 reads are clean
                    nc.sync.dma_start(
                        out=pt_d[jc][:, a0:i0], in_=zero_s[:, 0 : P * r]
                    )
                starts = [i0] + list(range(a0 + 512, T, 512))
                nch = len(starts)
                zp = zp_pool.tile([P, NIB], F32, tag="zp", name="zp")
                for ci, a in enumerate(starts):
                    b = a0 + 512 * (ci + 1)
                    w = b - a
                    ps = s_ps.tile([P, 512], F32, tag="sps", name="ps_s")
                    for kk in range(NKK):
                        nc.tensor.matmul(
                            ps[:, 0:w],
                            lhsT=kt_s[kk][:, _ts(jc, P)],
                            rhs=qt_s[kk][:, a:b],
                            start=(kk == 0),
                            stop=(kk == NKK - 1),
                        )
                    if ci == 0:
                        nc.vector.tensor_add(ps[:, 0:P], ps[:, 0:P], mask_s)
                    ptb = ptb_pool.tile([P, 512], F16, tag="ptb", name="ptb")
                    nc.scalar.activation(
                        ptb[:, 0:w],
                        ps[:, 0:w],
                        mybir.ActivationFunctionType.Exp,
                        bias=expshift_s[:, 0:1],
                        scale=1.0,
                        accum_out=zp[:, ci : ci + 1],
                    )
                    nc.sync.dma_start(out=pt_d[jc][:, a:b], in_=ptb[:, 0:w])
                z = zp_pool.tile([P, 1], F32, tag="zf", name="z")
                nc.vector.reduce_sum(z, zp[:, 0:nch], axis=mybir.AxisListType.X)
                nc.vector.reciprocal(zr_s[:, jc : jc + 1], z)
                # fold 1/Z_j into V rows (partition j)
                nc.vector.tensor_scalar_mul(v_s[jc], v_s[jc], zr_s[:, jc : jc + 1])
            s_ps_cm.__exit__(None, None, None)

            # ---- Phase 2: act^T = V'^T @ P~ ----
            act_ps = ctx.enter_context(
                tc.tile_pool(name="act_ps", bufs=1, space="PSUM")
            )
            pti_pool = ctx.enter_context(tc.tile_pool(name="pti", bufs=6))
            ob_pool = ctx.enter_context(tc.tile_pool(name="ob", bufs=4))
            for ib in range(NIB):
                njc = 4 * (ib + 1)
                pss = [
                    act_ps.tile([P, 512], F32, tag=f"aps{v}", name=f"aps{v}")
                    for v in range(4)
                ]
                for jc in range(njc):
                    pti = pti_pool.tile([P, 512], F16, tag="pti", name="pti")
                    nc.sync.dma_start(out=pti, in_=pt_d[jc][:, _ts(ib, 512)])
                    for vc in range(4):
                        nc.tensor.matmul(
                            pss[vc],
                            lhsT=v_s[jc][:, _ts(vc, P)],
                            rhs=pti,
                            start=(jc == 0),
                            stop=(jc == njc - 1),
                        )
                for vc in range(4):
                    ob = ob_pool.tile([P, 512], F32, tag="ob", name="ob")
                    if vc % 2 == 0:
                        nc.scalar.copy(ob, pss[vc])
                    else:
                        nc.vector.tensor_copy(ob, pss[vc])
                    nc.sync.dma_start(
                        out=out_d[C + vc * P : C + (vc + 1) * P, _ts(ib, 512)],
                        in_=ob,
                    )

    nc.compile()
    return nc


def _host_inputs(x, Wq, bq, Wk, bk, Wv, bv):
    c4 = float(C) ** 0.25
    wqt = np.ascontiguousarray(Wq.T / c4).astype(np.float16)
    wkt = np.ascontiguousarray(Wk.T / c4).astype(np.float16)
    wvt = np.ascontiguousarray(Wv.T).astype(np.float16)
    bq_h = np.ascontiguousarray((bq / c4).reshape(NKK, P).T).astype(np.float32)
    bk_h = np.ascontiguousarray((bk / c4).reshape(NKK, P).T).astype(np.float32)
    bv_h = np.ascontiguousarray(np.tile(bv.astype(np.float32), (P, 1)))
    r = np.arange(P)
    mask = np.where(r[None, :] >= r[:, None], 0.0, MASK_NEG).astype(np.float32)
    in_maps = []
    for b in range(x.shape[0]):
        xb = np.ascontiguousarray(x[b]).astype(np.float32)
        in_maps.append(
            {
                "x16": xb.astype(np.float16),
                "x32": xb,
                "wqt": wqt,
                "wkt": wkt,
                "wvt": wvt,
                "bq": bq_h,
                "bk": bk_h,
                "bv": bv_h,
                "mask": mask,
            }
        )
    return in_maps


def kernel(x, Wq, bq, Wk, bk, Wv, bv, _trace=False):
    import time as _time

    x = np.asarray(x, dtype=np.float32)
    if "nc" not in _CACHE:
        t0 = _time.time()
        _CACHE["nc"] = build_nc()
        print(f"[kernel] build_nc done in {_time.time() - t0:.1f}s", flush=True)
    nc = _CACHE["nc"]
    in_maps = _host_inputs(
        x,
        np.asarray(Wq, np.float32),
        np.asarray(bq, np.float32),
        np.asarray(Wk, np.float32),
        np.asarray(bk, np.float32),
        np.asarray(Wv, np.float32),
        np.asarray(bv, np.float32),
    )
    t0 = _time.time()
    res = run_bass_kernel_spmd(
        nc, in_maps, core_ids=list(range(8)), trace=_trace
    )
    print(f"[kernel] run done in {_time.time() - t0:.1f}s", flush=True)
    _CACHE["last_result"] = res
    out = np.stack([r["out"] for r in res.results]).astype(np.float32)
    return out
